# revision 15
# baseline (speedup 1.0000x reference)
"""Trainium2 Bass kernel for nn_Actor_67422396612916 (GNN message passing).

Data-parallel over batch B=16 across 8 NeuronCores (2 batches/core).
Per batch (N=1024 nodes, E=4 edge types folded to one adjacency sum):
    adj_s = adj_raw[..., 1:].sum(-1)              (N, N)
    h1 = node @ W1 + b1 ; h1 = adj_s @ h1 + h1    (N, 64)
    h2 = h1 @ W2 + b2   ; h  = adj_s @ h2 + h2    (N, 32)
    x = [h, node] ; gate = sig(x@Ws+bs)*tanh(x@Wt+bt)
    g = tanh(sum_n gate) ; MLP ; out = g @ Wl + bl  (16,)

v4 design notes (on top of v3):
  - adj is cast to fp16 on the host -> HBM stream halves to ~21MB/core.
    All conv-path tensors (S', node, h1, h1', h2) are fp16 in SBUF with
    fp32 PSUM accumulation; only |h| overflows fp16 so the conv2 output
    is stored as h*2^-5 and 2^5 is folded into the gate weights Wsh/Wth.
  - Residual (+I) is folded into S' = S + I via a tiny DVE add of the
    identity block on the diagonal of each S^T row-tile; all residual
    identity matmuls disappear.
  - conv1/h2 run per 2-tile pair (256-wide); conv2 accumulates
    incrementally into a held PSUM tile as (j-block, 256-col-chunk)
    operands become available, so only the last pair's contributions
    remain after the stream ends.
  - Gate PSUMs (psg/pst, 4 banks) are seeded with the node-part matmuls
    mid-stream; the tail only adds the h-part and runs
    sigmoid*tanh -> fused multiply-reduce.
  - Weight/node DMAs ride the scalar HWDGE ring so the sync ring carries
    only the adjacency stream; per-batch outs are written at the very
    end to keep the sync FIFO unblocked.
  - Last pair of the last batch streams as 2 single-tile DMAs so the
    edge-sum chain starts at the half-way point.
"""

import os
import sys

import numpy as np

if "/opt/trn_rl_repo" not in sys.path:
    sys.path.insert(0, "/opt/trn_rl_repo")

B, N, C = 16, 1024, 32          # batch, nodes, node feat
GC1, GC2 = 64, 32
AUX = 128
Z = 16
NCORES = 8
BPC = B // NCORES               # batches per core
P = 128                         # partition tile
NT = N // P                     # row tiles per batch (8)
NPAIR = NT // 2                 # 2-tile pairs (4)
HSCALE = 2.0 ** -5              # h (conv2 out) fp16 storage scale

_STATE = {}


def _build():
    import concourse.mybir as mybir
    import concourse.tile as tile
    from concourse import bacc
    from contextlib import ExitStack

    f32 = mybir.dt.float32
    f16 = mybir.dt.float16
    AF = mybir.ActivationFunctionType

    nc = bacc.Bacc(
        "TRN2",
        target_bir_lowering=False,
        debug=False,
        enable_asserts=False,
        num_devices=NCORES,
    )

    node_e = nc.declare_dram_parameter("node", [BPC, N, C], f16, isOutput=False)
    adj_e = nc.declare_dram_parameter("adj_raw", [BPC, N, N, 5], f16, isOutput=False)
    WH = GC1 + GC2 + 4 * AUX + P          # fp16 pack columns
    WF = 128 + 128 + Z + 1 + 1 + Z        # fp32 pack columns
    wph_e = nc.declare_dram_parameter("wpackh", [P, WH], f16, isOutput=False)
    wpf_e = nc.declare_dram_parameter("wpackf", [P, WF], f32, isOutput=False)
    out_e = nc.declare_dram_parameter("out", [BPC, Z], f32, isOutput=True)

    with tile.TileContext(nc) as tc, ExitStack() as ctx:
        const = ctx.enter_context(tc.tile_pool(name="const", bufs=1))
        wph = const.tile([P, WH], f16)
        nc.scalar.dma_start(out=wph[:], in_=wph_e.ap())
        wpf = const.tile([P, WF], f32)
        nc.scalar.dma_start(out=wpf[:], in_=wpf_e.ap())
        o = [0]

        def _col(width, rows, t):
            c0 = o[0]
            o[0] += width
            return t[0:rows, c0 : c0 + width]

        w1a = _col(GC1, C + 1, wph)
        w2a = _col(GC2, GC1 + 1, wph)
        wsna = _col(AUX, C + 1, wph)
        wsh = _col(AUX, GC2, wph)
        wtna = _col(AUX, C + 1, wph)
        wth = _col(AUX, GC2, wph)
        identh = _col(P, P, wph)
        o = [0]
        wm1 = _col(128, AUX, wpf)
        wm2 = _col(128, 128, wpf)
        wl = _col(Z, 128, wpf)
        bm1 = _col(1, 128, wpf)
        bm2 = _col(1, 128, wpf)
        blr = _col(Z, 1, wpf)

        rawp = ctx.enter_context(tc.tile_pool(name="raw", bufs=4))
        uvp = ctx.enter_context(tc.tile_pool(name="uv", bufs=2))
        sp = ctx.enter_context(tc.tile_pool(name="s", bufs=2))
        stp = ctx.enter_context(tc.tile_pool(name="st", bufs=2))
        hp = ctx.enter_context(tc.tile_pool(name="h", bufs=2))
        augp = ctx.enter_context(tc.tile_pool(name="aug", bufs=2))
        gsb = ctx.enter_context(tc.tile_pool(name="gsb", bufs=2))
        smp = ctx.enter_context(tc.tile_pool(name="sm", bufs=2))
        obp = ctx.enter_context(tc.tile_pool(name="ob", bufs=1))

        # PSUM budget is 8 banks of 2KB/partition; every pool uses ONE
        # bank-sized tag (allocation is bank-granular per tag per buf):
        #   ps_tr 2 + ps_mm 2 + ps_c2 2 + ps_g 2 = 8
        ps_tr = ctx.enter_context(tc.tile_pool(name="ps_tr", bufs=2, space="PSUM"))
        ps_mm = ctx.enter_context(tc.tile_pool(name="ps_mm", bufs=2, space="PSUM"))
        ps_c2 = ctx.enter_context(tc.tile_pool(name="ps_c2", bufs=1, space="PSUM"))
        ps_g = ctx.enter_context(tc.tile_pool(name="ps_g", bufs=2, space="PSUM"))

        # ---------- prelude: node path + h1 for BOTH batches ----------
        ntas, h1_alls = [], []
        for b in range(BPC):
            node_nat = hp.tile([P, NT, C], f16, tag="node_nat")
            nc.scalar.dma_start(
                out=node_nat[:],
                in_=node_e.ap()[b].rearrange("(t p) c -> p t c", p=P),
            )
            nta = augp.tile([C + 1, N], f16, tag="nta")        # [node^T; ones]
            nc.gpsimd.memset(nta[C : C + 1, :], 1.0)
            for t in range(NT):
                tr = ps_tr.tile([P, 512], f16, tag="pt")
                nc.tensor.transpose(tr[0:C, 0:P], node_nat[:, t, :], identh[:])
                nc.scalar.copy(nta[0:C, t * P : (t + 1) * P], tr[0:C, 0:P])

            # h1^T = (node @ W1 + b1)^T : (64, 1024), fp16
            h1t = hp.tile([GC1, N], f16, tag="h1t")
            for cc in range(4):
                psh = ps_mm.tile([P, 256], f32, tag="mm")
                nc.tensor.matmul(
                    psh[0:GC1, :], lhsT=w1a[:], rhs=nta[:, cc * 256 : (cc + 1) * 256]
                )
                nc.scalar.copy(h1t[:, cc * 256 : (cc + 1) * 256], psh[0:GC1, :])
            # h1 natural tiles (stationary operand for conv1)
            h1_all = hp.tile([P, NT, GC1], f16, tag="h1_all")
            for t in range(NT):
                tr = ps_tr.tile([P, 512], f16, tag="pt")
                nc.tensor.transpose(
                    tr[0:P, 0:GC1], h1t[:, t * P : (t + 1) * P], identh[0:GC1, 0:GC1]
                )
                nc.scalar.copy(h1_all[:, t, :], tr[0:P, 0:GC1])
            ntas.append(nta)
            h1_alls.append(h1_all)

        for b in range(BPC):
            nta, h1_all = ntas[b], h1_alls[b]
            st_t = stp.tile([P, NT, N], f16, tag="st")   # S'^T: [m, j, n]
            h1pta = augp.tile([GC1 + 1, N], f16, tag="h1pta")  # [h1'^T; ones]
            nc.gpsimd.memset(h1pta[GC1 : GC1 + 1, :], 1.0)
            h2t = hp.tile([GC2, N], f16, tag="h2t")
            h2_all = hp.tile([P, NT, GC2], f16, tag="h2_all")
            ht = augp.tile([GC2, N], f16, tag="ht")      # (h * 2^-5)^T
            psc2 = ps_c2.tile([GC2, 4, 256], f32, tag="c2")
            # conv2 (j, q) emission order; one PSUM accumulation group per
            # bank (q-pair): start on first MM into the bank, stop on last
            c2_sched = [
                (j, q)
                for p_ in range(NPAIR)
                for j in range(NT)
                for q in range(4)
                if max(j // 2, q) == p_
            ]
            c2_first = {bk: next(i for i, (j, q) in enumerate(c2_sched) if q // 2 == bk)
                        for bk in range(2)}
            c2_last = {bk: max(i for i, (j, q) in enumerate(c2_sched) if q // 2 == bk)
                       for bk in range(2)}
            c2_idx = [0]

            for p in range(NPAIR):
                raw = rawp.tile([P, 2, N, 5], f16, tag="raw")
                src = adj_e.ap()[b, p * 2 * P : (p + 1) * 2 * P].rearrange(
                    "(t p) n e -> p t n e", p=P
                )
                if p == NPAIR - 1:
                    # split: edge-sum of tile 0 overlaps tile 1's DMA
                    nc.sync.dma_start(out=raw[:, 0:1], in_=src[:, 0:1])
                    nc.sync.dma_start(out=raw[:, 1:2], in_=src[:, 1:2])
                else:
                    nc.sync.dma_start(out=raw[:], in_=src)
                w = uvp.tile([P, 2, N, 2], f16, tag="w")
                s = sp.tile([P, 2, N], f16, tag="s")
                for t in range(2):
                    i = 2 * p + t
                    nc.vector.tensor_add(
                        w[:, t], raw[:, t, :, 1:3], raw[:, t, :, 3:5]
                    )
                    nc.gpsimd.tensor_add(s[:, t], w[:, t, :, 0], w[:, t, :, 1])
                    for hblk in range(2):
                        pt = ps_tr.tile([P, 4 * P], f16, tag="pt")
                        for q in range(4):
                            j = 4 * hblk + q
                            nc.tensor.transpose(
                                pt[:, q * P : (q + 1) * P],
                                s[:, t, j * P : (j + 1) * P],
                                identh[:],
                            )
                        nc.scalar.copy(
                            st_t[:, 4 * hblk : 4 * hblk + 4, i * P : (i + 1) * P],
                            pt[:, 0 : 4 * P].rearrange("p (j n) -> p j n", j=4),
                        )
                    # fold the residual: S' = S + I on the diagonal block
                    nc.vector.tensor_add(
                        st_t[:, i, i * P : (i + 1) * P],
                        st_t[:, i, i * P : (i + 1) * P],
                        identh[:],
                    )

                cols = slice(p * 2 * P, (p + 1) * 2 * P)
                # conv1 on this 256-col chunk (sum over all j row-blocks)
                ps1 = ps_mm.tile([P, 256], f32, tag="mm")
                for j in range(NT):
                    nc.tensor.matmul(
                        ps1[0:GC1, :],
                        lhsT=h1_all[:, j, :],
                        rhs=st_t[:, j, cols],
                        start=(j == 0),
                        stop=(j == NT - 1),
                    )
                nc.scalar.copy(h1pta[0:GC1, cols], ps1[0:GC1, :])
                # h2 chunk
                psh2 = ps_mm.tile([P, 256], f32, tag="mm")
                nc.tensor.matmul(psh2[0:GC2, :], lhsT=w2a[:], rhs=h1pta[:, cols])
                nc.scalar.copy(h2t[:, cols], psh2[0:GC2, :])
                for j in (2 * p, 2 * p + 1):
                    tr = ps_tr.tile([P, 512], f16, tag="pt")
                    nc.tensor.transpose(
                        tr[0:P, 0:GC2], h2t[:, j * P : (j + 1) * P], identh[0:GC2, 0:GC2]
                    )
                    nc.scalar.copy(h2_all[:, j, :], tr[0:P, 0:GC2])

                # conv2: accumulate every (j-block, col-chunk) that just
                # became ready:  ready(j, q) at pair max(j//2, q)
                for j in range(NT):
                    for q in range(4):
                        if max(j // 2, q) != p:
                            continue
                        i_mm = c2_idx[0]
                        c2_idx[0] += 1
                        bk = q // 2
                        nc.tensor.matmul(
                            psc2[:, q, :],
                            lhsT=h2_all[:, j, :],
                            rhs=st_t[:, j, q * 256 : (q + 1) * 256],
                            start=(i_mm == c2_first[bk]),
                            stop=(i_mm == c2_last[bk]),
                        )

            # ---- tail: gates + reduce + MLP (per 256-col chunk) ----
            # ht copies are bank-wide (q-pair) reads so ScalarE never
            # touches a psc2 bank TensorE is still accumulating into
            # (PSUM bank collisions are fatal on HW).
            nc.scalar.activation(
                ht[:, 0:512], psc2[:, 0:2, :], AF.Copy, scale=HSCALE
            )
            nc.scalar.activation(
                ht[:, 512:N], psc2[:, 2:4, :], AF.Copy, scale=HSCALE
            )
            sg = gsb.tile([AUX, N], f16, tag="sg")
            tg = gsb.tile([AUX, N], f16, tag="tg")
            prod = gsb.tile([AUX, N], f16, tag="prod")
            for q in range(4):
                qs = slice(q * 256, (q + 1) * 256)
                # psg/pst from a 2-buf pool -> distinct banks, so the
                # sigmoid read never collides with the pst matmul write
                gqt = ps_g.tile([AUX, 512], f32, tag="g")
                tqt = ps_g.tile([AUX, 512], f32, tag="g")
                psgq = gqt[:, 0:256]
                pstq = tqt[:, 0:256]
                nc.tensor.matmul(
                    psgq, lhsT=wsna[:], rhs=nta[:, qs], start=True, stop=False
                )
                nc.tensor.matmul(
                    psgq, lhsT=wsh[:], rhs=ht[:, qs], start=False, stop=True
                )
                nc.tensor.matmul(
                    pstq, lhsT=wtna[:], rhs=nta[:, qs], start=True, stop=False
                )
                nc.tensor.matmul(
                    pstq, lhsT=wth[:], rhs=ht[:, qs], start=False, stop=True
                )
                nc.scalar.activation(sg[:, qs], psgq, AF.Sigmoid)
                nc.scalar.activation(tg[:, qs], pstq, AF.Tanh)
                nc.vector.tensor_mul(prod[:, qs], sg[:, qs], tg[:, qs])
            r_c = smp.tile([AUX, 1], f32, tag="r_c")
            nc.vector.reduce_sum(r_c[:], prod[:], axis=mybir.AxisListType.X)
            gcol = smp.tile([AUX, 1], f32, tag="gcol")
            nc.scalar.activation(gcol[:], r_c[:], AF.Tanh)

            # MLP head (fp32)
            pm = ps_mm.tile([P, 256], f32, tag="mm")
            nc.tensor.matmul(pm[:, 0:1], lhsT=wm1[:], rhs=gcol[:])
            g1 = smp.tile([128, 1], f32, tag="g1")
            nc.scalar.activation(g1[:], pm[:, 0:1], AF.Tanh, bias=bm1[:])
            pm2 = ps_mm.tile([P, 256], f32, tag="mm")
            nc.tensor.matmul(pm2[:, 0:1], lhsT=wm2[:], rhs=g1[:])
            g2 = smp.tile([128, 1], f32, tag="g2")
            nc.scalar.activation(g2[:], pm2[:, 0:1], AF.Tanh, bias=bm2[:])
            pm3 = ps_mm.tile([P, 256], f32, tag="mm")
            nc.tensor.matmul(pm3[0:1, 0:Z], lhsT=g2[:], rhs=wl[:])
            p3 = pm3[0:1, 0:Z]
            ob = obp.tile([1, Z], f32, tag=f"ob{b}")
            nc.vector.tensor_add(ob[:], p3[:], blr[:])
            _STATE.setdefault("obs", []).append((b, ob))

        # outs at the very end so the sync ring never blocks mid-stream
        for b, ob in _STATE.pop("obs"):
            nc.sync.dma_start(out=out_e.ap()[b : b + 1, :], in_=ob[:])

    nc.finalize()
    return nc


def _prep_weights(inputs):
    f = np.float32
    h = np.float16
    W1, b1 = inputs["W1"], inputs["b1"]
    W2, b2 = inputs["W2"], inputs["b2"]
    Ws, bs = inputs["Ws"], inputs["bs"]
    Wt, bt = inputs["Wt"], inputs["bt"]
    Wm1, bm1 = inputs["Wm1"], inputs["bm1"]
    Wm2, bm2 = inputs["Wm2"], inputs["bm2"]
    Wl, bl = inputs["Wl"], inputs["bl"]
    WH = GC1 + GC2 + 4 * AUX + P
    WF = 128 + 128 + Z + 1 + 1 + Z
    inv = 1.0 / HSCALE           # 2^5, folded into the gate h-weights

    def put(dst, a, o):
        r, w_ = a.shape
        dst[0:r, o : o + w_] = a
        return o + w_

    wph = np.zeros((P, WH), h)
    o = 0
    o = put(wph, np.concatenate([W1, b1[None, :]], 0).astype(h), o)
    o = put(wph, np.concatenate([W2, b2[None, :]], 0).astype(h), o)
    o = put(wph, np.concatenate([Ws[GC2 : GC2 + C], bs[None, :]], 0).astype(h), o)
    o = put(wph, (Ws[0:GC2] * inv).astype(h), o)
    o = put(wph, np.concatenate([Wt[GC2 : GC2 + C], bt[None, :]], 0).astype(h), o)
    o = put(wph, (Wt[0:GC2] * inv).astype(h), o)
    o = put(wph, np.eye(P, dtype=h), o)
    assert o == WH
    wpf = np.zeros((P, WF), f)
    o = 0
    o = put(wpf, Wm1.astype(f), o)
    o = put(wpf, Wm2.astype(f), o)
    o = put(wpf, Wl.astype(f), o)
    o = put(wpf, bm1.reshape(128, 1).astype(f), o)
    o = put(wpf, bm2.reshape(128, 1).astype(f), o)
    o = put(wpf, bl.reshape(1, Z).astype(f), o)
    assert o == WF
    return {"wpackh": wph, "wpackf": wpf}


def _ensure_ntff_hook():
    """The agent image's antenv lacks axon_hooks; bass_utils imports it
    unconditionally on the trace path. Shim it and register the ctypes
    NTFF hook against the axon PJRT .so."""
    import types

    try:
        from antenv.axon_hooks import get_axon_ntff_profile_hook  # noqa: F401

        return
    except ImportError:
        pass
    holder = {}
    mod = types.ModuleType("antenv.axon_hooks")
    mod.set_axon_ntff_profile_hook = lambda hk: holder.update(h=hk)
    mod.get_axon_ntff_profile_hook = lambda: holder.get("h")
    sys.modules["antenv.axon_hooks"] = mod
    import antenv

    antenv.axon_hooks = mod
    so_path = "/opt/axon/libaxon_pjrt.so"
    if os.path.exists(so_path):
        from trn_agent_boot.trn_boot import _ntff_profile_via_ctypes

        mod.set_axon_ntff_profile_hook(_ntff_profile_via_ctypes(so_path))


def kernel(**inputs):
    inputs = {k: np.asarray(v) for k, v in inputs.items()}
    node = inputs["node"].astype(np.float16)
    adj = inputs["adj_raw"].astype(np.float16)
    shared = _prep_weights(inputs)

    if "nc" not in _STATE:
        _STATE["nc"] = _build()
    nc = _STATE["nc"]

    in_maps = []
    for i in range(NCORES):
        m = dict(shared)
        m["node"] = np.ascontiguousarray(node[i * BPC : (i + 1) * BPC])
        m["adj_raw"] = np.ascontiguousarray(adj[i * BPC : (i + 1) * BPC])
        in_maps.append(m)

    if os.environ.get("KERNEL_SIM") == "1":
        from concourse import bass_interp

        sim = bass_interp.MultiCoreSim(nc, 1)
        for k, vv in in_maps[0].items():
            sim.cores[0].tensor(k)[:] = vv
        sim.simulate()
        out0 = np.array(sim.cores[0].mem_tensor("out"))
        full = np.zeros((B, Z), np.float32)
        full[0:BPC] = out0
        _STATE["exec_ns"] = None
        return full

    from concourse.bass_utils import run_bass_kernel_spmd

    trace = os.environ.get("KERNEL_TRACE") == "1"
    if trace:
        _ensure_ntff_hook()
    tmpdir = os.environ.get("KERNEL_TMPDIR")
    res = run_bass_kernel_spmd(
        nc, in_maps, core_ids=list(range(NCORES)), trace=trace, tmpdir=tmpdir
    )
    _STATE["exec_ns"] = res.exec_time_ns
    _STATE["res"] = res
    out = np.concatenate([res.results[i]["out"] for i in range(NCORES)], 0)
    return out.astype(np.float32)


def last_exec_time_ns():
    return _STATE.get("exec_ns")


# revision 20
# speedup vs baseline: 1.0258x; 1.0258x over previous
"""Trainium2 Bass kernel for nn_Actor_67422396612916 (GNN message passing).

Data-parallel over batch B=16 across 8 NeuronCores (2 batches/core).
Per batch (N=1024 nodes, E=4 edge types folded to one adjacency sum):
    adj_s = adj_raw[..., 1:].sum(-1)              (N, N)
    h1 = node @ W1 + b1 ; h1 = adj_s @ h1 + h1    (N, 64)
    h2 = h1 @ W2 + b2   ; h  = adj_s @ h2 + h2    (N, 32)
    x = [h, node] ; gate = sig(x@Ws+bs)*tanh(x@Wt+bt)
    g = tanh(sum_n gate) ; MLP ; out = g @ Wl + bl  (16,)

v4 design notes (on top of v3):
  - adj is cast to fp16 on the host -> HBM stream halves to ~21MB/core.
    All conv-path tensors (S', node, h1, h1', h2) are fp16 in SBUF with
    fp32 PSUM accumulation; only |h| overflows fp16 so the conv2 output
    is stored as h*2^-5 and 2^5 is folded into the gate weights Wsh/Wth.
  - Residual (+I) is folded into S' = S + I via a tiny DVE add of the
    identity block on the diagonal of each S^T row-tile; all residual
    identity matmuls disappear.
  - conv1/h2 run per 2-tile pair (256-wide); conv2 accumulates
    incrementally into a held PSUM tile as (j-block, 256-col-chunk)
    operands become available, so only the last pair's contributions
    remain after the stream ends.
  - Gate PSUMs (psg/pst, 4 banks) are seeded with the node-part matmuls
    mid-stream; the tail only adds the h-part and runs
    sigmoid*tanh -> fused multiply-reduce.
  - Weight/node DMAs ride the scalar HWDGE ring so the sync ring carries
    only the adjacency stream; per-batch outs are written at the very
    end to keep the sync FIFO unblocked.
  - Last pair of the last batch streams as 2 single-tile DMAs so the
    edge-sum chain starts at the half-way point.
"""

import os
import sys

import numpy as np

if "/opt/trn_rl_repo" not in sys.path:
    sys.path.insert(0, "/opt/trn_rl_repo")

B, N, C = 16, 1024, 32          # batch, nodes, node feat
GC1, GC2 = 64, 32
AUX = 128
Z = 16
NCORES = 8
BPC = B // NCORES               # batches per core
P = 128                         # partition tile
NT = N // P                     # row tiles per batch (8)
NPAIR = NT // 2                 # 2-tile pairs (4)
HSCALE = 2.0 ** -5              # h (conv2 out) fp16 storage scale

_STATE = {}


def _build():
    import concourse.mybir as mybir
    import concourse.tile as tile
    from concourse import bacc
    from contextlib import ExitStack

    f32 = mybir.dt.float32
    f16 = mybir.dt.float16
    AF = mybir.ActivationFunctionType

    nc = bacc.Bacc(
        "TRN2",
        target_bir_lowering=False,
        debug=False,
        enable_asserts=False,
        num_devices=NCORES,
    )

    node_e = nc.declare_dram_parameter("node", [BPC, N, C], f16, isOutput=False)
    adj_e = nc.declare_dram_parameter("adj_raw", [BPC, N, N, 5], f16, isOutput=False)
    WH = GC1 + GC2 + 4 * AUX + P          # fp16 pack columns
    WF = 128 + 128 + Z + 1 + 1 + Z        # fp32 pack columns
    wph_e = nc.declare_dram_parameter("wpackh", [P, WH], f16, isOutput=False)
    wpf_e = nc.declare_dram_parameter("wpackf", [P, WF], f32, isOutput=False)
    out_e = nc.declare_dram_parameter("out", [BPC, Z], f32, isOutput=True)

    with tile.TileContext(nc) as tc, ExitStack() as ctx:
        const = ctx.enter_context(tc.tile_pool(name="const", bufs=1))
        wph = const.tile([P, WH], f16)
        nc.scalar.dma_start(out=wph[:], in_=wph_e.ap())
        wpf = const.tile([P, WF], f32)
        nc.scalar.dma_start(out=wpf[:], in_=wpf_e.ap())
        o = [0]

        def _col(width, rows, t):
            c0 = o[0]
            o[0] += width
            return t[0:rows, c0 : c0 + width]

        w1a = _col(GC1, C + 1, wph)
        w2a = _col(GC2, GC1 + 1, wph)
        wsna = _col(AUX, C + 1, wph)
        wsh = _col(AUX, GC2, wph)
        wtna = _col(AUX, C + 1, wph)
        wth = _col(AUX, GC2, wph)
        identh = _col(P, P, wph)
        o = [0]
        wm1 = _col(128, AUX, wpf)
        wm2 = _col(128, 128, wpf)
        wl = _col(Z, 128, wpf)
        bm1 = _col(1, 128, wpf)
        bm2 = _col(1, 128, wpf)
        blr = _col(Z, 1, wpf)

        rawp = ctx.enter_context(tc.tile_pool(name="raw", bufs=4))
        uvp = ctx.enter_context(tc.tile_pool(name="uv", bufs=2))
        sp = ctx.enter_context(tc.tile_pool(name="s", bufs=2))
        stp = ctx.enter_context(tc.tile_pool(name="st", bufs=2))
        hp = ctx.enter_context(tc.tile_pool(name="h", bufs=2))
        augp = ctx.enter_context(tc.tile_pool(name="aug", bufs=2))
        gsb = ctx.enter_context(tc.tile_pool(name="gsb", bufs=2))
        smp = ctx.enter_context(tc.tile_pool(name="sm", bufs=2))
        obp = ctx.enter_context(tc.tile_pool(name="ob", bufs=1))

        # PSUM budget is 8 banks of 2KB/partition; every pool uses ONE
        # bank-sized tag (allocation is bank-granular per tag per buf):
        #   ps_tr 2 + ps_mm 2 + ps_c2 4 = 8
        ps_tr = ctx.enter_context(tc.tile_pool(name="ps_tr", bufs=2, space="PSUM"))
        ps_mm = ctx.enter_context(tc.tile_pool(name="ps_mm", bufs=2, space="PSUM"))
        ps_c2 = ctx.enter_context(tc.tile_pool(name="ps_c2", bufs=2, space="PSUM"))

        # ---------- prelude: node path + h1 for BOTH batches ----------
        ntas, h1_alls = [], []
        for b in range(BPC):
            node_nat = hp.tile([P, NT, C], f16, tag="node_nat")
            nc.scalar.dma_start(
                out=node_nat[:],
                in_=node_e.ap()[b].rearrange("(t p) c -> p t c", p=P),
            )
            nta = augp.tile([C + 1, N], f16, tag="nta")        # [node^T; ones]
            nc.gpsimd.memset(nta[C : C + 1, :], 1.0)
            for t in range(NT):
                tr = ps_tr.tile([P, 512], f16, tag="pt")
                nc.tensor.transpose(tr[0:C, 0:P], node_nat[:, t, :], identh[:])
                nc.scalar.copy(nta[0:C, t * P : (t + 1) * P], tr[0:C, 0:P])

            # h1 natural tiles (stationary operand for conv1), computed
            # directly per 128-node block: h1[blk] = [node|1][blk] @ [W1;b1]
            h1_all = hp.tile([P, NT, GC1], f16, tag="h1_all")
            for t in range(NT):
                psh = ps_mm.tile([P, 256], f32, tag="mm")
                nc.tensor.matmul(
                    psh[0:P, 0:GC1],
                    lhsT=nta[:, t * P : (t + 1) * P],
                    rhs=w1a[:],
                )
                nc.scalar.copy(h1_all[:, t, :], psh[0:P, 0:GC1])
            ntas.append(nta)
            h1_alls.append(h1_all)

        for b in range(BPC):
            nta, h1_all = ntas[b], h1_alls[b]
            st_t = stp.tile([P, NT, N], f16, tag="st")   # S'^T: [m, j, n]
            h1pta = augp.tile([GC1 + 1, N], f16, tag="h1pta")  # [h1'^T; ones]
            nc.gpsimd.memset(h1pta[GC1 : GC1 + 1, :], 1.0)
            h2_all = hp.tile([P, NT, GC2], f16, tag="h2_all")
            ht = augp.tile([GC2, N], f16, tag="ht")      # (h * 2^-5)^T
            psc2 = ps_c2.tile([GC2, 4, 256], f32, tag="c2")
            # conv2 (j, q) emission order; one PSUM accumulation group per
            # bank (q-pair): start on first MM into the bank, stop on last
            c2_sched = [
                (j, q)
                for p_ in range(NPAIR)
                for j in range(NT)
                for q in range(4)
                if max(j // 2, q) == p_
            ]
            c2_first = {bk: next(i for i, (j, q) in enumerate(c2_sched) if q // 2 == bk)
                        for bk in range(2)}
            c2_last = {bk: max(i for i, (j, q) in enumerate(c2_sched) if q // 2 == bk)
                       for bk in range(2)}
            c2_idx = [0]

            for p in range(NPAIR):
                raw = rawp.tile([P, 2, N, 5], f16, tag="raw")
                src = adj_e.ap()[b, p * 2 * P : (p + 1) * 2 * P].rearrange(
                    "(t p) n e -> p t n e", p=P
                )
                if p == NPAIR - 1:
                    # split: edge-sum of tile 0 overlaps tile 1's DMA
                    nc.sync.dma_start(out=raw[:, 0:1], in_=src[:, 0:1])
                    nc.sync.dma_start(out=raw[:, 1:2], in_=src[:, 1:2])
                else:
                    nc.sync.dma_start(out=raw[:], in_=src)
                w = uvp.tile([P, 2, N, 2], f16, tag="w")
                s = sp.tile([P, 2, N], f16, tag="s")
                if p == NPAIR - 1:
                    # per-tile adds so tile 0's chain overlaps tile 1's DMA
                    for t in range(2):
                        nc.vector.tensor_add(
                            w[:, t], raw[:, t, :, 1:3], raw[:, t, :, 3:5]
                        )
                        nc.vector.tensor_add(
                            s[:, t], w[:, t, :, 0], w[:, t, :, 1]
                        )
                else:
                    nc.vector.tensor_add(w[:], raw[:, :, :, 1:3], raw[:, :, :, 3:5])
                    nc.vector.tensor_add(s[:], w[:, :, :, 0], w[:, :, :, 1])
                for t in range(2):
                    i = 2 * p + t
                    for hblk in range(2):
                        pt = ps_tr.tile([P, 4 * P], f16, tag="pt")
                        for q in range(4):
                            j = 4 * hblk + q
                            nc.tensor.transpose(
                                pt[:, q * P : (q + 1) * P],
                                s[:, t, j * P : (j + 1) * P],
                                identh[:],
                            )
                        nc.scalar.copy(
                            st_t[:, 4 * hblk : 4 * hblk + 4, i * P : (i + 1) * P],
                            pt[:, 0 : 4 * P].rearrange("p (j n) -> p j n", j=4),
                        )
                    # fold the residual: S' = S + I on the diagonal block
                    nc.gpsimd.tensor_add(
                        st_t[:, i, i * P : (i + 1) * P],
                        st_t[:, i, i * P : (i + 1) * P],
                        identh[:],
                    )

                cols = slice(p * 2 * P, (p + 1) * 2 * P)
                # conv1 on this 256-col chunk (sum over all j row-blocks)
                ps1 = ps_mm.tile([P, 256], f32, tag="mm")
                for j in range(NT):
                    nc.tensor.matmul(
                        ps1[0:GC1, :],
                        lhsT=h1_all[:, j, :],
                        rhs=st_t[:, j, cols],
                        start=(j == 0),
                        stop=(j == NT - 1),
                    )
                nc.scalar.copy(h1pta[0:GC1, cols], ps1[0:GC1, :])
                # h2 natural per 128-node block: h2[blk] = [h1'|1][blk] @ [W2;b2]
                for t, j in ((0, 2 * p), (1, 2 * p + 1)):
                    psh2 = ps_mm.tile([P, 256], f32, tag="mm")
                    nc.tensor.matmul(
                        psh2[0:P, 0:GC2],
                        lhsT=h1pta[:, j * P : (j + 1) * P],
                        rhs=w2a[:],
                    )
                    nc.scalar.copy(h2_all[:, j, :], psh2[0:P, 0:GC2])

                # conv2: accumulate every (j-block, col-chunk) that just
                # became ready:  ready(j, q) at pair max(j//2, q)
                for j in range(NT):
                    for q in range(4):
                        if max(j // 2, q) != p:
                            continue
                        i_mm = c2_idx[0]
                        c2_idx[0] += 1
                        bk = q // 2
                        nc.tensor.matmul(
                            psc2[:, q, :],
                            lhsT=h2_all[:, j, :],
                            rhs=st_t[:, j, q * 256 : (q + 1) * 256],
                            start=(i_mm == c2_first[bk]),
                            stop=(i_mm == c2_last[bk]),
                        )

            # ---- tail: gates + reduce + MLP (per 256-col chunk) ----
            # ht copies are bank-wide (q-pair) reads so ScalarE never
            # touches a psc2 bank TensorE is still accumulating into
            # (PSUM bank collisions are fatal on HW).
            nc.scalar.activation(
                ht[:, 0:512], psc2[:, 0:2, :], AF.Copy, scale=HSCALE
            )
            nc.scalar.activation(
                ht[:, 512:N], psc2[:, 2:4, :], AF.Copy, scale=HSCALE
            )
            sg = gsb.tile([AUX, N], f16, tag="sg")
            tg = gsb.tile([AUX, N], f16, tag="tg")
            prod = gsb.tile([AUX, N], f16, tag="prod")
            for q in range(4):
                qs = slice(q * 256, (q + 1) * 256)
                # psg/pst from the rotating 2-buf pool -> distinct banks, so
                # the sigmoid read never collides with the pst matmul write
                gqt = ps_mm.tile([P, 256], f32, tag="mm")
                tqt = ps_mm.tile([P, 256], f32, tag="mm")
                psgq = gqt[:, :]
                pstq = tqt[:, :]
                nc.tensor.matmul(
                    psgq, lhsT=wsna[:], rhs=nta[:, qs], start=True, stop=False
                )
                nc.tensor.matmul(
                    psgq, lhsT=wsh[:], rhs=ht[:, qs], start=False, stop=True
                )
                nc.tensor.matmul(
                    pstq, lhsT=wtna[:], rhs=nta[:, qs], start=True, stop=False
                )
                nc.tensor.matmul(
                    pstq, lhsT=wth[:], rhs=ht[:, qs], start=False, stop=True
                )
                nc.scalar.activation(sg[:, qs], psgq, AF.Sigmoid)
                nc.scalar.activation(tg[:, qs], pstq, AF.Tanh)
                nc.vector.tensor_mul(prod[:, qs], sg[:, qs], tg[:, qs])
            r_c = smp.tile([AUX, 1], f32, tag="r_c")
            nc.vector.reduce_sum(r_c[:], prod[:], axis=mybir.AxisListType.X)
            gcol = smp.tile([AUX, 1], f32, tag="gcol")
            nc.scalar.activation(gcol[:], r_c[:], AF.Tanh)

            # MLP head (fp32)
            pm = ps_mm.tile([P, 256], f32, tag="mm")
            nc.tensor.matmul(pm[:, 0:1], lhsT=wm1[:], rhs=gcol[:])
            g1 = smp.tile([128, 1], f32, tag="g1")
            nc.scalar.activation(g1[:], pm[:, 0:1], AF.Tanh, bias=bm1[:])
            pm2 = ps_mm.tile([P, 256], f32, tag="mm")
            nc.tensor.matmul(pm2[:, 0:1], lhsT=wm2[:], rhs=g1[:])
            g2 = smp.tile([128, 1], f32, tag="g2")
            nc.scalar.activation(g2[:], pm2[:, 0:1], AF.Tanh, bias=bm2[:])
            pm3 = ps_mm.tile([P, 256], f32, tag="mm")
            nc.tensor.matmul(pm3[0:1, 0:Z], lhsT=g2[:], rhs=wl[:])
            p3 = pm3[0:1, 0:Z]
            ob = obp.tile([1, Z], f32, tag=f"ob{b}")
            nc.vector.tensor_add(ob[:], p3[:], blr[:])
            _STATE.setdefault("obs", []).append((b, ob))

        # outs at the very end so the sync ring never blocks mid-stream
        for b, ob in _STATE.pop("obs"):
            nc.sync.dma_start(out=out_e.ap()[b : b + 1, :], in_=ob[:])

    nc.finalize()
    return nc


def _prep_weights(inputs):
    f = np.float32
    h = np.float16
    W1, b1 = inputs["W1"], inputs["b1"]
    W2, b2 = inputs["W2"], inputs["b2"]
    Ws, bs = inputs["Ws"], inputs["bs"]
    Wt, bt = inputs["Wt"], inputs["bt"]
    Wm1, bm1 = inputs["Wm1"], inputs["bm1"]
    Wm2, bm2 = inputs["Wm2"], inputs["bm2"]
    Wl, bl = inputs["Wl"], inputs["bl"]
    WH = GC1 + GC2 + 4 * AUX + P
    WF = 128 + 128 + Z + 1 + 1 + Z
    inv = 1.0 / HSCALE           # 2^5, folded into the gate h-weights

    def put(dst, a, o):
        r, w_ = a.shape
        dst[0:r, o : o + w_] = a
        return o + w_

    wph = np.zeros((P, WH), h)
    o = 0
    o = put(wph, np.concatenate([W1, b1[None, :]], 0).astype(h), o)
    o = put(wph, np.concatenate([W2, b2[None, :]], 0).astype(h), o)
    o = put(wph, np.concatenate([Ws[GC2 : GC2 + C], bs[None, :]], 0).astype(h), o)
    o = put(wph, (Ws[0:GC2] * inv).astype(h), o)
    o = put(wph, np.concatenate([Wt[GC2 : GC2 + C], bt[None, :]], 0).astype(h), o)
    o = put(wph, (Wt[0:GC2] * inv).astype(h), o)
    o = put(wph, np.eye(P, dtype=h), o)
    assert o == WH
    wpf = np.zeros((P, WF), f)
    o = 0
    o = put(wpf, Wm1.astype(f), o)
    o = put(wpf, Wm2.astype(f), o)
    o = put(wpf, Wl.astype(f), o)
    o = put(wpf, bm1.reshape(128, 1).astype(f), o)
    o = put(wpf, bm2.reshape(128, 1).astype(f), o)
    o = put(wpf, bl.reshape(1, Z).astype(f), o)
    assert o == WF
    return {"wpackh": wph, "wpackf": wpf}


def _ensure_ntff_hook():
    """The agent image's antenv lacks axon_hooks; bass_utils imports it
    unconditionally on the trace path. Shim it and register the ctypes
    NTFF hook against the axon PJRT .so."""
    import types

    try:
        from antenv.axon_hooks import get_axon_ntff_profile_hook  # noqa: F401

        return
    except ImportError:
        pass
    holder = {}
    mod = types.ModuleType("antenv.axon_hooks")
    mod.set_axon_ntff_profile_hook = lambda hk: holder.update(h=hk)
    mod.get_axon_ntff_profile_hook = lambda: holder.get("h")
    sys.modules["antenv.axon_hooks"] = mod
    import antenv

    antenv.axon_hooks = mod
    so_path = "/opt/axon/libaxon_pjrt.so"
    if os.path.exists(so_path):
        from trn_agent_boot.trn_boot import _ntff_profile_via_ctypes

        mod.set_axon_ntff_profile_hook(_ntff_profile_via_ctypes(so_path))


def kernel(**inputs):
    inputs = {k: np.asarray(v) for k, v in inputs.items()}
    node = inputs["node"].astype(np.float16)
    adj = inputs["adj_raw"].astype(np.float16)
    shared = _prep_weights(inputs)

    if "nc" not in _STATE:
        _STATE["nc"] = _build()
    nc = _STATE["nc"]

    in_maps = []
    for i in range(NCORES):
        m = dict(shared)
        m["node"] = np.ascontiguousarray(node[i * BPC : (i + 1) * BPC])
        m["adj_raw"] = np.ascontiguousarray(adj[i * BPC : (i + 1) * BPC])
        in_maps.append(m)

    if os.environ.get("KERNEL_SIM") == "1":
        from concourse import bass_interp

        sim = bass_interp.MultiCoreSim(nc, 1)
        for k, vv in in_maps[0].items():
            sim.cores[0].tensor(k)[:] = vv
        sim.simulate()
        out0 = np.array(sim.cores[0].mem_tensor("out"))
        full = np.zeros((B, Z), np.float32)
        full[0:BPC] = out0
        _STATE["exec_ns"] = None
        return full

    from concourse.bass_utils import run_bass_kernel_spmd

    trace = os.environ.get("KERNEL_TRACE") == "1"
    if trace:
        _ensure_ntff_hook()
    tmpdir = os.environ.get("KERNEL_TMPDIR")
    res = run_bass_kernel_spmd(
        nc, in_maps, core_ids=list(range(NCORES)), trace=trace, tmpdir=tmpdir
    )
    _STATE["exec_ns"] = res.exec_time_ns
    _STATE["res"] = res
    out = np.concatenate([res.results[i]["out"] for i in range(NCORES)], 0)
    return out.astype(np.float32)


def last_exec_time_ns():
    return _STATE.get("exec_ns")
